# revision 41
# baseline (speedup 1.0000x reference)
"""Trainium2 Bass kernel for nn_BlastLinear (block low-rank linear layer).

Math (reference):
  y[q,n,r] = sum_c x[n, q*1024+c] * C[q,r,c]          (mm1, per input block q)
  z[p,n,r] = sum_q D[p,q,r] * y[q,n,r]                (tiny mix over q)
  o[p,n,j] = sum_r z[p,n,r] * B[p,j,r]                (mm2, per output block p)
  out[n, p*1024+j] = o[p,n,j] + bias[p*1024+j]

Sharding: pure data-parallel over the 8192 tokens -> 1024 tokens per core,
weights replicated, no collectives.

Precision/speed: both matmuls run as fp8(e4m3) DoubleRow matmuls on hi/lo
SPLIT operands. Each logical bf16 k-tile product (a+b)(c+d) is computed as
the three fp8 products HH+HL+LH (the lo*lo term is ~eps^2 and dropped),
and DoubleRow packs TWO fp8 k-tile products per PE instruction at 0.5
cycles/row -> 1.5 instructions per logical k-tile = 0.75x the bf16 PE
time with ~bf16 accuracy (measured rel err ~3.3e-3 vs the 2e-2 gate;
single non-split e4m3 anywhere measures ~2.3e-2 and fails).
e4m3's narrow range forces power-of-2 pre-scaling: C,B ship as x64,
the mix uses D/2 so z' = 32z (~N(0,0.8), clear of e4m3 subnormals),
and PSUM o = 2048*o drains raw to bf16; the host applies /2048 + bias.

Per-core structure (chunk = 512 tokens, 2 chunks, PE order
mm1(c0), mm1(c1), mm2(c0), mm2(c1)):
  mm1:  psum y'[q,rt] [128r x 512n] += 12 DoubleRow matmuls per (q,rt)
        (classes HH/HL/LH over 4 k-tile pairs), x_hi/x_lo + C_hi/C_lo
        planes prepped on host.
  ycp:  yb = bf16(y') PSUM->SBUF on ACT
  mix:  zb[p,rt] += (D/2)[p,q,rt]*yb[q,rt]: mul (DVE 4x bf16) + adds
        (q1,q3 on DVE 2x; q2 on Pool)
  zsplit: zh = e4m3(zb) on ACT; zl = e4m3(zb - zh) (DVE on chunk 0,
        Pool on later chunks - keeps DVE ahead of mm2's deadline)
  mm2:  psum o[g] [128o x 512n] += 6 DoubleRow matmuls per (p,ot)
        (BH*zh, BL*zh, BH*zl over 2 rt-pairs)
  drain: raw copy PSUM->SBUF bf16 rotated over ACT/DVE/Pool; out ships
        bf16 TRANSPOSED [OUT_F, n_core] = 2048*o; host scales + bias.
DMA notes: ~25 MiB/core at ~360 GB/s effective sits under the 82us PE
floor; x ships as two fp8 planes (same bytes as bf16), C/B as hi/lo fp8
planes (same bytes as bf16). Batched transfers as in the bf16 baseline;
B prefetches during the last mm1 chunk; tail out-DMAs split small.
"""

import numpy as np

import concourse.mybir as mybir
import concourse.tile as tile
from concourse import bacc
from concourse.bass_utils import run_bass_kernel_spmd

N_CORES = 8
IN_F = 4096
OUT_F = 4096
P = 4
Q = 4
R = 512
CB = IN_F // Q        # 1024 input features per q block
OB = OUT_F // P       # 1024 output features per p block
N_TOK = 4 * 2048      # 8192 total tokens
N_CORE = N_TOK // N_CORES   # 1024 tokens per core

CHUNK = 512           # tokens per pipeline chunk
KT1 = CB // 128       # 8 contraction tiles per q in mm1
RT = R // 128         # 4 rank partition tiles
KB = 4                # k-tiles per x DMA batch
OG = 4                # o-groups per out DMA batch
OT = OB // 128        # 8 output-feature tiles per p

W_SCALE = 64.0        # C,B host pre-scale (keeps e4m3 operands normal)
Z_SCALE = 32.0        # z' = 32*z via D' = D*(Z_SCALE/W_SCALE)
O_SCALE = W_SCALE * Z_SCALE  # psum o = 2048*o; host divides

F32 = mybir.dt.float32
BF16 = mybir.dt.bfloat16
FP8 = mybir.dt.float8e4
ADD = mybir.AluOpType.add
SUB = mybir.AluOpType.subtract
DR = mybir.MatmulPerfMode.DoubleRow

_cached_nc = None


def _build(n_core=N_CORE, chunk=CHUNK):
    nc = bacc.Bacc("TRN2", target_bir_lowering=False, debug=False,
                   enable_asserts=False)

    def din(name, shape, dtype):
        return nc.dram_tensor(name, shape, dtype, kind="ExternalInput").ap()

    nb = (n_core // chunk) * Q * (KT1 // KB)
    # x pre-tiled on host into per-batch SBUF images [128, KB, 2, chunk]
    xt_d = din("xt", [nb, 128, KB, 2, chunk], FP8)
    # C^T / B^T with hi/lo planes interleaved per k-tile: [p, t, w, cols]
    c_d = din("c", [128, IN_F // 128, 2, R], FP8)
    b_d = din("b", [128, (P * R) // 128, 2, OB], FP8)
    dr = din("dr", [R, P * Q], F32)
    outT = nc.dram_tensor("outT", [OUT_F, n_core], BF16,
                          kind="ExternalOutput").ap()

    n_chunks = n_core // chunk

    with tile.TileContext(nc) as tc:
        with (
            tc.tile_pool(name="const", bufs=1) as cpool,
            tc.tile_pool(name="xp", bufs=4) as xpool,
            tc.tile_pool(name="ybp", bufs=8) as ybpool,
            tc.tile_pool(name="tp", bufs=3) as tpool,
            tc.tile_pool(name="zbp", bufs=3 * n_chunks + 1) as zbpool,
            tc.tile_pool(name="zhp", bufs=2 * P * n_chunks) as zhpool,
            tc.tile_pool(name="outp", bufs=3) as outpool,
            tc.tile_pool(name="yps", bufs=2, space="PSUM") as ypool,
            tc.tile_pool(name="ops", bufs=2, space="PSUM") as opool,
        ):
            # C^T planes [p, k-tile, hi/lo, R]; one DMA moves both
            # planes of a k-tile batch (inner (2,R) is contiguous)
            c_sb = cpool.tile([128, IN_F // 128, 2, R], FP8)
            # B^T planes [p, r-tile, hi/lo, OB]
            b_sb = cpool.tile([128, (P * R) // 128, 2, OB], FP8)
            # d_sb[r_, rt, p*4+q] = (D/2)[p, q, rt*128 + r_]
            d_sb = cpool.tile([128, RT, P * Q], F32)

            zb = {}   # (j, p) -> z' bf16 plane [128, RT, chunk]
            zh = {}   # (j, p) -> [128, RT, chunk] fp8 hi plane
            zl = {}   # (j, p) -> [128, RT, chunk] fp8 lo plane
            pending_splits = []

            def emit_split(j, p, on_act):
                zh[(j, p)] = zhpool.tile([128, RT, chunk], FP8, tag="zh",
                                         name=f"zh_{j}_{p}")
                zl[(j, p)] = zhpool.tile([128, RT, chunk], FP8, tag="zl",
                                         name=f"zl_{j}_{p}")
                zp = zb[(j, p)]
                if on_act:
                    nc.scalar.copy(zh[(j, p)][:], zp[:])
                else:
                    nc.gpsimd.tensor_copy(zh[(j, p)][:], zp[:])
                # last chunk: mm2 is waiting on these planes, so the lo
                # subtract pipelines on DVE (free after the mix) behind
                # Pool's hi copy; earlier chunks keep Pool for both (DVE
                # still owns the next chunk's mix)
                seng = nc.vector if j == n_chunks - 1 else nc.gpsimd
                seng.tensor_tensor(
                    zl[(j, p)][:], zp[:], zh[(j, p)][:], op=SUB)

            def flush_splits():
                while pending_splits:
                    emit_split(*pending_splits.pop(0), on_act=True)

            def emit_mm1(j):
                last_c = j == n_chunks - 1
                for q in range(Q):
                    # two 2-bank PSUM pair tiles per q (rt 0,1 / 2,3)
                    ysp = [
                        ypool.tile([128, 2, chunk], F32, tag="y",
                                   name=f"y_{j}_{q}_{h}")
                        for h in range(RT // 2)
                    ]
                    for kb in range(KT1 // KB):
                        base_t = q * KT1 + kb * KB
                        first = j == 0 and q == 0 and kb == 0
                        x_t = xpool.tile([128, KB, 2, chunk], FP8, tag="x",
                                         name=f"x_{j}_{q}_{kb}")
                        bi = (j * Q + q) * (KT1 // KB) + kb

                        def xdma(w, lo, hi):
                            nc.sync.dma_start(
                                x_t[:, lo:hi, w, :],
                                xt_d[bi, :, lo:hi, w, :])

                        def xdma2(lo, hi):
                            # both planes of the k-batch in one transfer
                            nc.sync.dma_start(
                                x_t[:, lo:hi, :, :],
                                xt_d[bi, :, lo:hi, :, :])

                        def cdma(w, lo, hi):
                            hs = slice(base_t + lo, base_t + hi)
                            nc.sync.dma_start(c_sb[:, hs, w, :],
                                              c_d[:, hs, w, :])

                        def cdma2(lo, hi):
                            hs = slice(base_t + lo, base_t + hi)
                            nc.sync.dma_start(c_sb[:, hs, :, :],
                                              c_d[:, hs, :, :])

                        if first:
                            # cold start: first k-pair's hi pieces ship
                            # alone so the first matmul waits on ~320KiB,
                            # then lo planes + rest stream under compute
                            cdma(0, 0, 2)
                            xdma(0, 0, 2)
                            cdma(1, 0, 2)
                            xdma(1, 0, 2)
                            cdma2(2, KB)
                            xdma2(2, KB)
                        else:
                            if j == 0:
                                cdma2(0, KB)
                            xdma2(0, KB)
                        if j == 0 and q == 0 and kb == 1:
                            nc.sync.dma_start(
                                d_sb[:],
                                dr.rearrange("(t p) s -> p t s", p=128))
                        if j == n_chunks - 1:
                            # B stays out of the x stream except p0's
                            # warmup pieces in the last two slots; the
                            # rest ships during mm2(c0). n_chunks==1
                            # loads everything here.
                            idx = q * 2 + kb
                            if n_chunks == 1:
                                bs = slice(idx * 2, idx * 2 + 2)
                                nc.sync.dma_start(b_sb[:, bs, :, :],
                                                  b_d[:, bs, :, :])
                            elif idx == 7:
                                nc.sync.dma_start(b_sb[:, 0:2, :, :],
                                                  b_d[:, 0:2, :, :])
                        # 3 fp8 product classes x 2 k-pairs x 4 rt tiles;
                        # class-major so the xl tiles are needed last
                        n_batches = KT1 // KB
                        # PSUM-pair-major emission: pair h's stops land
                        # ~6 instructions before the batch end, so its
                        # drain overlaps the other pair's matmuls and the
                        # next q's first matmuls aren't gated on a drain
                        for h in range(RT // 2):
                            for ci, (cw, xw) in enumerate(
                                    ((0, 0), (1, 0), (0, 1))):
                                for kp in range(KB // 2):
                                    for rt in (2 * h, 2 * h + 1):
                                        ks = slice(base_t + 2 * kp,
                                                   base_t + 2 * kp + 2)
                                        ms = slice(2 * kp, 2 * kp + 2)
                                        nc.tensor.matmul(
                                            ysp[h][:, rt % 2, :],
                                            lhsT=c_sb[:, ks, cw,
                                                      rt * 128:
                                                      (rt + 1) * 128],
                                            rhs=x_t[:, ms, xw, :],
                                            start=(kb == 0 and ci == 0
                                                   and kp == 0),
                                            stop=(kb == n_batches - 1
                                                  and ci == 2
                                                  and kp == KB // 2 - 1),
                                            perf_mode=DR)
                    # y' -> SBUF bf16 on ACT: one fused 2-bank drain per
                    # PSUM pair (1038ns vs 2x612)
                    ybp = []
                    for h in range(RT // 2):
                        yb_t = ybpool.tile([128, 2, chunk], BF16, tag="yb",
                                           name=f"yb_{j}_{q}_{h}")
                        nc.scalar.copy(yb_t[:], ysp[h][:])
                        ybp.append(yb_t)

                    def yb_ap(rt):
                        return ybp[rt // 2][:, rt % 2, :]

                    # mix entirely on DVE (mul 4x bf16, adds 2x bf16):
                    # Pool's slow TT ops stay OUT of the q1->q2->q3 chain.
                    # p-major so p0's z plane completes first and mm2 can
                    # start on it while later p's still mix.
                    for p in range(P):
                        if q == 0 and (j, p) not in zb:
                            zb[(j, p)] = zbpool.tile(
                                [128, RT, chunk], BF16, tag="zb",
                                name=f"zb_{j}_{p}")
                        zp = zb[(j, p)]
                        for rt in range(RT):
                            col = p * Q + q
                            dcol = d_sb[:, rt, col:col + 1]
                            if q == 0:
                                nc.vector.tensor_scalar_mul(
                                    zp[:, rt, :], yb_ap(rt), dcol)
                            else:
                                tt = tpool.tile([128, chunk], BF16, tag="t",
                                                name=f"t_{j}_{q}_{p}_{rt}")
                                nc.vector.tensor_scalar_mul(
                                    tt[:], yb_ap(rt), dcol)
                                nc.vector.tensor_tensor(
                                    zp[:, rt, :], tt[:], zp[:, rt, :],
                                    op=ADD)
                        if q == Q - 1:
                            # whole-plane hi/lo split ([128, RT*chunk]
                            # ops), both on Pool: splits on ACT head-of-
                            # line block later y/out drains, and deferring
                            # them holds zb planes the next chunk needs
                            emit_split(j, p, on_act=False)

            def emit_mm2(j):
                ob_t = None
                osp = None
                last = j == n_chunks - 1
                NG = P * OT
                if j == 0 and n_chunks > 1:
                    # finish B(p0) right before mm2 starts consuming it
                    nc.sync.dma_start(b_sb[:, 2:4, :, :], b_d[:, 2:4, :, :])
                for p in range(P):
                    for ot in range(OT):
                        g = p * OT + ot
                        if j == 0 and n_chunks > 1 and p < P - 1 and ot < 2:
                            # stream B(p+1) hi/lo while mm2 runs on p
                            bs = slice((p + 1) * RT + 2 * ot,
                                       (p + 1) * RT + 2 * ot + 2)
                            nc.sync.dma_start(b_sb[:, bs, :, :],
                                              b_d[:, bs, :, :])
                        if g % 2 == 0:
                            osp = opool.tile([128, 2, chunk], F32, tag="o",
                                             name=f"o_{j}_{g}")
                        ops = osp[:, g % 2, :]
                        # 3 fp8 classes x 2 rt-pairs of DoubleRow matmuls
                        n_i = 0
                        for bw, mt in ((0, zh[(j, p)]), (1, zh[(j, p)]),
                                       (0, zl[(j, p)])):
                            for rp in range(RT // 2):
                                rs = slice(p * RT + 2 * rp,
                                           p * RT + 2 * rp + 2)
                                ms = slice(2 * rp, 2 * rp + 2)
                                nc.tensor.matmul(
                                    ops,
                                    lhsT=b_sb[:, rs, bw,
                                              ot * 128:(ot + 1) * 128],
                                    rhs=mt[:, ms, :],
                                    start=(n_i == 0), stop=(n_i == 5),
                                    perf_mode=DR)
                                n_i += 1
                        if g % OG == 0:
                            ob_t = outpool.tile([128, OG, chunk], BF16,
                                                tag="ob", name=f"ob_{j}_{g}")
                        if g % 2 == 0:
                            continue
                        # drain one 2-bank PSUM pair (groups g-1, g) in a
                        # single fused op; raw bf16 copies, scale/bias on
                        # host. Early chunks all on ACT; last chunk
                        # rotates ACT/DVE/Pool.
                        s0 = (g % OG) - 1
                        dst = ob_t[:, s0:s0 + 2, :]
                        if last and g >= NG - 2:
                            # final pair: halves on ACT and DVE in
                            # parallel, each group its own DMA
                            nc.scalar.copy(ob_t[:, s0, :], osp[:, 0, :])
                            nc.vector.tensor_copy(ob_t[:, s0 + 1, :],
                                                  osp[:, 1, :])
                            for gg in (g - 1, g):
                                nc.sync.dma_start(
                                    outT[gg * 128:(gg + 1) * 128,
                                         j * chunk:(j + 1) * chunk],
                                    ob_t[:, s0 + gg - g + 1, :])
                            continue
                        # Pool runs the z-splits. DVE still owns the
                        # later chunks' mix during mm2(c0), so early
                        # chunks drain on ACT alone; the last chunk
                        # alternates ACT/DVE.
                        if j < n_chunks - 1:
                            eng = nc.scalar
                        else:
                            eng = (nc.scalar, nc.vector)[(g // 2) % 2]
                        if eng is nc.scalar:
                            nc.scalar.copy(dst, osp[:])
                        else:
                            eng.tensor_copy(dst, osp[:])
                        if last and g == NG - 3:
                            # penultimate pair ships immediately as its
                            # own DMA so the tail stays short
                            nc.sync.dma_start(
                                outT[(g - 1) * 128:(g + 1) * 128,
                                     j * chunk:(j + 1) * chunk]
                                .rearrange("(t p) n -> p t n", p=128),
                                dst)
                        elif last and g >= NG - 8 and g % 2 == 1:
                            # end-of-kernel: ship each drained pair at
                            # once so the final DMAs are small and early
                            nc.sync.dma_start(
                                outT[(g - 1) * 128:(g + 1) * 128,
                                     j * chunk:(j + 1) * chunk]
                                .rearrange("(t p) n -> p t n", p=128),
                                dst)
                        elif g % OG == OG - 1:
                            nc.sync.dma_start(
                                outT[(g - OG + 1) * 128:(g + 1) * 128,
                                     j * chunk:(j + 1) * chunk]
                                .rearrange("(t p) n -> p t n", p=128),
                                ob_t[:])

            for j in range(n_chunks):
                emit_mm1(j)
            for j in range(n_chunks):
                emit_mm2(j)

    nc.compile()
    return nc


def _split8(a):
    import ml_dtypes
    e4 = ml_dtypes.float8_e4m3
    hi = np.ascontiguousarray(a).astype(e4)
    lo = np.ascontiguousarray(a - hi.astype(np.float32)).astype(e4)
    return hi, lo


def _tile_w(hi, lo, ob):
    """[T, ob] hi/lo planes -> [128, T//128, 2, ob] (plane inner)."""
    T = hi.shape[0]
    a = np.stack([hi, lo])                       # [2, T, ob]
    a = a.reshape(2, T // 128, 128, ob)          # [w, t, p, ob]
    return np.ascontiguousarray(a.transpose(2, 1, 0, 3))


def _tile_x(xh, xl, n_core):
    """[IN_F, n_core] planes -> per-batch tiles [nb,128,KB,2,chunk]."""
    nch = n_core // CHUNK
    a = np.stack([xh, xl])                       # [w, IN_F, n]
    a = a.reshape(2, IN_F // 128, 128, nch, CHUNK)  # [w, t, p, j, n]
    # batch bi = (j*Q + q)*2 + kb covers t = q*8 + kb*4 + kk
    a = a.reshape(2, Q, KT1 // KB, KB, 128, nch, CHUNK)
    a = a.transpose(5, 1, 2, 4, 3, 0, 6)         # [j, q, kb, p, kk, w, n]
    return np.ascontiguousarray(
        a.reshape(nch * Q * (KT1 // KB), 128, KB, 2, CHUNK))


def _prep_in_maps(x, B, C, D, bias):
    x2 = np.asarray(x, dtype=np.float32).reshape(N_TOK, IN_F)
    CT = np.ascontiguousarray(
        (np.asarray(C, dtype=np.float32) * W_SCALE)
        .transpose(0, 2, 1).reshape(IN_F, R))
    c_t = _tile_w(*_split8(CT), R)
    BT = np.ascontiguousarray(
        (np.asarray(B, dtype=np.float32) * W_SCALE)
        .transpose(0, 2, 1).reshape(P * R, OB))
    b_t = _tile_w(*_split8(BT), OB)
    DRm = np.ascontiguousarray(
        (np.asarray(D, dtype=np.float32) * (Z_SCALE / W_SCALE))
        .transpose(2, 0, 1).reshape(R, P * Q))

    in_maps = []
    for c in range(N_CORES):
        xt = np.ascontiguousarray(x2[c * N_CORE:(c + 1) * N_CORE].T)
        xh_c, xl_c = _split8(xt)
        in_maps.append({
            "xt": _tile_x(xh_c, xl_c, N_CORE),
            "c": c_t, "b": b_t, "dr": DRm,
        })
    return in_maps


def _run(in_maps, trace=False):
    global _cached_nc
    if _cached_nc is None:
        _cached_nc = _build()
    import time
    for attempt in range(3):
        try:
            return run_bass_kernel_spmd(
                _cached_nc, in_maps, list(range(N_CORES)), trace=trace)
        except Exception:
            # transient device errors (e.g. NRT_EXEC_UNIT_UNRECOVERABLE
            # from a previously wedged core) usually clear on retry
            if attempt == 2:
                raise
            time.sleep(5.0 * (attempt + 1))


def kernel(x, B, C, D, bias):
    xa = np.asarray(x)
    lead = xa.shape[:-1]
    biasf = np.asarray(bias, dtype=np.float32)
    res = _run(_prep_in_maps(x, B, C, D, bias))
    outs = [
        np.asarray(res.results[c]["outT"]).astype(np.float32).T
        * (1.0 / O_SCALE) + biasf
        for c in range(N_CORES)
    ]
    return np.concatenate(outs, axis=0).reshape(*lead, OUT_F)
